# revision 17
# baseline (speedup 1.0000x reference)
"""Trainium2 8-core Bass kernel for nn_Decoder_Layer_37177236914647.

Decoder layer: self-MHA(+causal mask) -> +res -> LN -> cross-MHA -> +res -> LN
-> FFN(2x dense, no act) -> +res -> LN.  Softmax is over the BATCH axis
(axis=0), faithful to the original model: w[b,h,q,k] = exp(s_b)/sum_b' exp(s_b').
With the reference's fp32 "+ mask*-1e9" the masked positions collapse to
exactly 0.25 for every batch (|scores| << ulp(1e9)=64), reproduced here with a
blend E' = E*(1-m) + m before the batch normalization.

Sharding: attention is head-parallel (16 heads / 8 cores = 2 heads per core;
the batch softmax is local per head).  Activations stay feature-major
([features, tokens]) so head shards concatenate on the partition axis:
  - LN1: partial (sum, sumsq) over the core's 128 features -> AllReduce 32KB.
  - normalized a (bf16) AllGather'd for the cross-attention Q projection.
  - AllToAll turns the feature-sharded attn2+res into token-sharded rows for
    the FFN (512 tokens/core, full weights), LN2/LN3 local.
Output returned token-sharded, reassembled and transposed on host.
"""
import numpy as np
import ml_dtypes

import concourse.bass as bass
import concourse.mybir as mybir
from concourse import bacc
import concourse.tile as tile
from concourse import bass_utils

NC = 8          # cores
B = 4           # batch
S = 1024        # seq len
D = 1024        # d_model
H = 16          # heads
HD = 64         # head dim
F = 128         # features per core (2 heads * 64)
T = B * S       # 4096 flattened tokens
TC = T // NC    # 512 tokens per core (FFN row shard)
NT = T // 512   # 8 token tiles of 512
NF = D // 128   # 8 feature tiles of 128
EPS = 1e-3
P = 128

FP32 = mybir.dt.float32
BF16 = mybir.dt.bfloat16
AX = mybir.AluOpType
AF = mybir.ActivationFunctionType

CLEAN, BOUNDARY, MASKED = 0, 1, 2
_LAST_NC = None
_LAST_IN_MAPS = None


def _emit(nc, tc, io, cls1, bidx):
    from contextlib import ExitStack

    n_bnd = max(bidx.values()) + 1 if bidx else 0
    ctx = ExitStack()
    with ctx:
        # ---- pools (bufs is per-tag N-buffering) ----
        wts = ctx.enter_context(tc.tile_pool(name="wts", bufs=3))
        srcp = ctx.enter_context(tc.tile_pool(name="srcp", bufs=9))
        scr = ctx.enter_context(tc.tile_pool(name="scr", bufs=4))
        x3fp = ctx.enter_context(tc.tile_pool(name="x3fp", bufs=8))
        wff = ctx.enter_context(tc.tile_pool(name="wff", bufs=2))
        acts = ctx.enter_context(tc.tile_pool(name="acts", bufs=3))
        epool = ctx.enter_context(tc.tile_pool(name="epool", bufs=3))
        wpool = ctx.enter_context(tc.tile_pool(name="wpool", bufs=2))
        drp = ctx.enter_context(tc.tile_pool(name="drp", bufs=2))
        big = ctx.enter_context(tc.tile_pool(name="big", bufs=2))
        smal = ctx.enter_context(tc.tile_pool(name="smal", bufs=1))
        lns = ctx.enter_context(tc.tile_pool(name="lns", bufs=1))
        abp = ctx.enter_context(tc.tile_pool(name="abp", bufs=1))
        ps = ctx.enter_context(tc.tile_pool(name="ps", bufs=4, space="PSUM"))
        pssc = ctx.enter_context(tc.tile_pool(name="pssc", bufs=2, space="PSUM"))
        dram = ctx.enter_context(tc.tile_pool(name="dram", bufs=1, space="DRAM"))

        # ---- constants ----
        ones_col = smal.tile([P, 1], FP32, tag="onesc")
        nc.vector.memset(ones_col[:], 1.0)
        ones_row = smal.tile([1, P], FP32, tag="onesr")
        nc.vector.memset(ones_row[:], 1.0)
        quarter = smal.tile([P, 512], BF16, tag="quart")
        nc.vector.memset(quarter[:], 0.25)
        eps_col = smal.tile([P, 1], FP32, tag="epsc")
        nc.vector.memset(eps_col[:], EPS)
        eps_row = smal.tile([1, 1], FP32, tag="epsr")
        nc.vector.memset(eps_row[:], EPS)
        zero_col = smal.tile([P, 1], FP32, tag="zeroc")
        nc.vector.memset(zero_col[:], 0.0)
        zero_row = smal.tile([1, 1], FP32, tag="zeror")
        nc.vector.memset(zero_row[:], 0.0)

        if n_bnd:
            m_sb = smal.tile([P, n_bnd * 512], BF16, tag="m")
            mb_sb = smal.tile([P, n_bnd * 512], BF16, tag="mb")
            nc.sync.dma_start(m_sb[:], io["mbnd"][:])
            nc.sync.dma_start(mb_sb[:], io["mbndbar"][:])

        def load_w(name, dt=FP32):
            w = wts.tile([P, NF * 128], dt, tag="w")
            nc.sync.dma_start(w[:, :].rearrange("p (f m) -> p f m", f=NF),
                              io[name].rearrange("(f p) m -> p f m", p=P))
            return w

        def projections(src_ap, w_list, out_dts, has_v):
            """src_ap: [D, T] dram.  w_list: list of weight sbuf tiles; the
            last one is the V weight if has_v.  Returns per-weight outputs:
            QK-style [P, T] and V token-major [P, 32*128]."""
            outs = []
            for wi, (w, dt) in enumerate(zip(w_list, out_dts)):
                outs.append(acts.tile([P, T], dt, tag="act",
                                      name=f"proj_out{wi}"))
            for j in range(NT):
                src = []
                for f in range(NF):
                    tl = srcp.tile([P, 512], src_ap.dtype, tag="xsrc")
                    nc.sync.dma_start(
                        tl[:], src_ap[f * 128:(f + 1) * 128,
                                      j * 512:(j + 1) * 512])
                    src.append(tl)
                nqk = len(w_list) - 1 if has_v else len(w_list)
                for wi in range(nqk):
                    pt = ps.tile([P, 512], FP32, tag="ps512")
                    for f in range(NF):
                        nc.tensor.matmul(
                            pt[:], w_list[wi][:, f * 128:(f + 1) * 128],
                            src[f][:], start=(f == 0), stop=(f == NF - 1))
                    nc.scalar.copy(outs[wi][:, j * 512:(j + 1) * 512], pt[:])
                if has_v:
                    wv = w_list[-1]
                    vout = outs[-1]
                    for i4 in range(4):
                        i = j * 4 + i4
                        pt = ps.tile([P, 512], FP32, tag="ps512")
                        for f in range(NF):
                            nc.tensor.matmul(
                                pt[:, :128],
                                src[f][:, i4 * 128:(i4 + 1) * 128],
                                wv[:, f * 128:(f + 1) * 128],
                                start=(f == 0), stop=(f == NF - 1))
                        nc.vector.tensor_copy(
                            vout[:, i * 128:(i + 1) * 128], pt[:, :128])
            return outs

        def attn(QT, KT, V, cls, x_out, res_ap, res_is_sbuf):
            """x_out[:, 1024b+512j : +512] = (sum_k W*V) + res, both heads."""
            for j in range(2):
                ot = [ps.tile([P, 512], FP32, tag="ps512", name=f"ot{b_}")
                      for b_ in range(4)]
                for t in range(8):
                    tile_cls = cls[t][j]
                    Wt = None
                    if tile_cls != MASKED:
                        Eh = [epool.tile([P, 4 * 512], BF16, tag="E",
                                          name=f"E{h_}") for h_ in range(2)]
                        for bp in range(2):
                            pt = [pssc.tile([P, 1024], FP32, tag="sc",
                                            name=f"sc{h_}") for h_ in range(2)]
                            for bi in range(2):
                                b = bp * 2 + bi
                                for hh in range(2):
                                    nc.tensor.matmul(
                                        pt[hh][:, bi * 512:(bi + 1) * 512],
                                        KT[64 * hh:64 * (hh + 1),
                                           1024 * b + 128 * t:
                                           1024 * b + 128 * (t + 1)],
                                        QT[64 * hh:64 * (hh + 1),
                                           1024 * b + 512 * j:
                                           1024 * b + 512 * (j + 1)],
                                        start=True, stop=True)
                            for hh in range(2):
                                nc.scalar.activation(
                                    Eh[hh][:, bp * 1024:(bp + 1) * 1024],
                                    pt[hh][:], AF.Exp, bias=zero_col[:])
                        if tile_cls == BOUNDARY:
                            sl = bidx[(t, j)]
                            mb = mb_sb[:, sl * 512:(sl + 1) * 512]
                            mm = m_sb[:, sl * 512:(sl + 1) * 512]
                            for hh in range(2):
                                ev = Eh[hh][:, :].rearrange(
                                    "p (b q) -> p b q", b=4)
                                nc.gpsimd.tensor_tensor(
                                    ev, ev,
                                    mb[:, None, :].broadcast_to([P, 4, 512]),
                                    op=AX.mult)
                                nc.gpsimd.tensor_tensor(
                                    ev, ev,
                                    mm[:, None, :].broadcast_to([P, 4, 512]),
                                    op=AX.add)
                        Wt = wpool.tile([P, 2, 4 * 512], BF16, tag="W")
                        for hh in range(2):
                            e3 = Eh[hh][:, :].rearrange("p (c q) -> p c q", c=4)
                            d2 = drp.tile([P, 2 * 512], BF16, tag="d2")
                            nc.vector.tensor_tensor(
                                d2[:, :].rearrange("p (c q) -> p c q", c=2),
                                e3[:, 0:2, :], e3[:, 2:4, :], op=AX.add)
                            dd = drp.tile([P, 512], BF16, tag="dd")
                            nc.vector.tensor_tensor(
                                dd[:], d2[:, 0:512], d2[:, 512:1024], op=AX.add)
                            rr = drp.tile([P, 512], BF16, tag="rr")
                            with nc.allow_low_precision(
                                    reason="softmax denom ~4.0, bf16 ok"):
                                nc.vector.reciprocal(rr[:], dd[:])
                            nc.vector.tensor_tensor(
                                Wt[:, hh, :].rearrange("p (b q) -> p b q", b=4),
                                e3[:, :, :],
                                rr[:, None, :].broadcast_to([P, 4, 512]),
                                op=AX.mult)
                    for b in range(4):
                        for hh in range(2):
                            if tile_cls == MASKED:
                                rhs = quarter[:, :]
                            else:
                                rhs = Wt[:, hh, b * 512:(b + 1) * 512]
                            nc.tensor.matmul(
                                ot[b][64 * hh:64 * (hh + 1), :],
                                V[:, 128 * (8 * b + t) + 64 * hh:
                                     128 * (8 * b + t) + 64 * (hh + 1)],
                                rhs, start=(t == 0), stop=(t == 7),
                                tile_position=(0, 64 * hh))
                for b in range(4):
                    sl = slice(1024 * b + 512 * j, 1024 * b + 512 * (j + 1))
                    if res_is_sbuf:
                        nc.vector.tensor_tensor(
                            x_out[:, sl], ot[b][:], res_ap[:, sl], op=AX.add)
                    else:
                        rt = scr.tile([P, 512], FP32, tag="scr")
                        nc.sync.dma_start(rt[:], res_ap[:, sl])
                        nc.vector.tensor_tensor(
                            x_out[:, sl], ot[b][:], rt[:], op=AX.add)

        # ================= MHA1 =================
        wq1 = load_w("wq1s")
        wk1 = load_w("wk1s")
        wv1 = load_w("wv1s")
        QT1, KT1, V1 = projections(io["xdT"], [wq1, wk1, wv1],
                                   [BF16, BF16, BF16], has_v=True)
        x1 = big.tile([P, T], FP32, tag="big")
        attn(QT1, KT1, V1, cls1, x1, io["xd_res"], False)

        # ================= LN1 (AllReduce stats) =================
        st_in = dram.tile([2 * T], FP32)
        st_out = dram.tile([2 * T], FP32, addr_space="Shared")
        for j in range(NT):
            sl = slice(j * 512, (j + 1) * 512)
            sq = scr.tile([P, 512], FP32, tag="scr")
            nc.vector.tensor_tensor(sq[:], x1[:, sl], x1[:, sl], op=AX.mult)
            p1 = ps.tile([1, 512], FP32, tag="ps512")
            nc.tensor.matmul(p1[:], ones_col[:], x1[:, sl], start=True, stop=True)
            p2 = ps.tile([1, 512], FP32, tag="ps512")
            nc.tensor.matmul(p2[:], ones_col[:], sq[:], start=True, stop=True)
            stj = abp.tile([1, 1024], FP32, tag="stj")
            nc.vector.tensor_copy(stj[:, 0:512], p1[:])
            nc.vector.tensor_copy(stj[:, 512:1024], p2[:])
            nc.sync.dma_start(st_in[512 * j:512 * (j + 1)].rearrange(
                "(o n) -> o n", o=1), stj[0:1, 0:512])
            nc.sync.dma_start(st_in[T + 512 * j:T + 512 * (j + 1)].rearrange(
                "(o n) -> o n", o=1), stj[0:1, 512:1024])
        nc.gpsimd.collective_compute(
            "AllReduce", AX.add, replica_groups=[list(range(NC))],
            ins=[st_in[:]], outs=[st_out[:]])

        s1r = smal.tile([P, 32], FP32, tag="s1r")
        s2r = smal.tile([P, 32], FP32, tag="s2r")
        nc.sync.dma_start(s1r[:], st_out[:T].rearrange("(p i) -> p i", p=P))
        nc.sync.dma_start(s2r[:], st_out[T:].rearrange("(p i) -> p i", p=P))
        mu = smal.tile([P, 32], FP32, tag="mu")
        nc.vector.tensor_scalar_mul(mu[:], s1r[:], 1.0 / D)
        mu2 = smal.tile([P, 32], FP32, tag="mu2")
        nc.vector.tensor_tensor(mu2[:], mu[:], mu[:], op=AX.mult)
        var = smal.tile([P, 32], FP32, tag="var")
        nc.vector.scalar_tensor_tensor(var[:], s2r[:], 1.0 / D, mu2[:],
                                       op0=AX.mult, op1=AX.subtract)
        nc.scalar.activation(var[:], var[:], AF.Ln, bias=eps_col[:])
        rr1 = smal.tile([P, 32], FP32, tag="rr1")
        nc.scalar.activation(rr1[:], var[:], AF.Exp, bias=zero_col[:], scale=-0.5)
        bneg = smal.tile([P, 32], FP32, tag="bneg")
        nc.vector.tensor_tensor(bneg[:], mu[:], rr1[:], op=AX.mult)

        ab_d = dram.tile([2, T], FP32)
        nc.sync.dma_start(ab_d[0, :].rearrange("(p i) -> p i", p=P), rr1[:])
        nc.sync.dma_start(ab_d[1, :].rearrange("(p i) -> p i", p=P), bneg[:])
        g1 = smal.tile([P, 1], FP32, tag="g1")
        be1 = smal.tile([P, 1], FP32, tag="be1")
        nc.sync.dma_start(g1[:], io["g1s"][:])
        nc.sync.dma_start(be1[:], io["be1s"][:])

        a_my = big.tile([P, T], FP32, tag="big")
        a_bf = acts.tile([P, T], BF16, tag="act")
        for j in range(NT):
            sl = slice(j * 512, (j + 1) * 512)
            a_row = abp.tile([1, 512], FP32, tag="abrow")
            b_row = abp.tile([1, 512], FP32, tag="abrow2")
            nc.sync.dma_start(a_row[:], ab_d[0:1, sl])
            nc.sync.dma_start(b_row[:], ab_d[1:2, sl])
            pra = ps.tile([P, 512], FP32, tag="ps512")
            nc.tensor.matmul(pra[:], ones_row[:], a_row[:],
                             start=True, stop=True)
            prb = ps.tile([P, 512], FP32, tag="ps512")
            nc.tensor.matmul(prb[:], ones_row[:], b_row[:],
                             start=True, stop=True)
            tt = scr.tile([P, 512], FP32, tag="scr")
            nc.vector.tensor_tensor(tt[:], x1[:, sl], pra[:], op=AX.mult)
            nc.vector.tensor_tensor(tt[:], tt[:], prb[:], op=AX.subtract)
            nc.scalar.activation(a_my[:, sl], tt[:], AF.Identity,
                                 bias=be1[:], scale=g1[:])
            nc.vector.tensor_copy(a_bf[:, sl], a_my[:, sl])

        ag_in = dram.tile([P, T], BF16)
        ag_out = dram.tile([D, T], BF16, addr_space="Shared")
        nc.sync.dma_start(ag_in[:], a_bf[:])
        nc.gpsimd.collective_compute(
            "AllGather", AX.bypass, replica_groups=[list(range(NC))],
            ins=[ag_in[:]], outs=[ag_out[:]])

        # ================= MHA2 =================
        wk2 = load_w("wk2s")
        wv2 = load_w("wv2s")
        KT2, V2 = projections(io["xeT"], [wk2, wv2], [BF16, BF16], has_v=True)
        wq2 = wts.tile([P, NF * 128], BF16, tag="w")
        nc.sync.dma_start(wq2[:, :].rearrange("p (f m) -> p f m", f=NF),
                          io["wq2s"].rearrange("(f p) m -> p f m", p=P))
        (QT2,) = projections(ag_out[:, :], [wq2], [BF16], has_v=False)

        cls_clean = [[CLEAN] * 2 for _ in range(8)]
        x2 = big.tile([P, T], FP32, tag="big")
        attn(QT2, KT2, V2, cls_clean, x2, a_my, True)

        # ================= A2A -> token shard =================
        a2a_in = dram.tile([D, TC], FP32)
        a2a_out = dram.tile([D, TC], FP32)
        nc.sync.dma_start(
            a2a_in[:, :].rearrange("(j p) q -> p j q", p=P),
            x2[:, :].rearrange("p (j q) -> p j q", j=NC))
        nc.gpsimd.collective_compute(
            "AllToAll", AX.bypass, replica_groups=[list(range(NC))],
            ins=[a2a_in[:]], outs=[a2a_out[:]])

        # ================= LN2 / FFN / LN3 (token shard) =================
        def ln_local(get_x, g_name, be_name, out_tile):
            sp1 = ps.tile([1, TC], FP32, tag="ps512")
            sp2 = ps.tile([1, TC], FP32, tag="ps512")
            for f in range(NF):
                xt = get_x(f)
                sq = scr.tile([P, TC], FP32, tag="scr")
                nc.vector.tensor_tensor(sq[:], xt[:], xt[:], op=AX.mult)
                nc.tensor.matmul(sp1[:], ones_col[:], xt[:],
                                 start=(f == 0), stop=(f == NF - 1))
                nc.tensor.matmul(sp2[:], ones_col[:], sq[:],
                                 start=(f == 0), stop=(f == NF - 1))
            mu_ = lns.tile([1, TC], FP32, tag="lmu")
            nc.vector.tensor_scalar_mul(mu_[:], sp1[:], 1.0 / D)
            mu2_ = lns.tile([1, TC], FP32, tag="lmu2")
            nc.vector.tensor_tensor(mu2_[:], mu_[:], mu_[:], op=AX.mult)
            var_ = lns.tile([1, TC], FP32, tag="lvar")
            nc.vector.scalar_tensor_tensor(var_[:], sp2[:], 1.0 / D, mu2_[:],
                                           op0=AX.mult, op1=AX.subtract)
            nc.scalar.activation(var_[:], var_[:], AF.Ln, bias=eps_row[:])
            rr_ = lns.tile([1, TC], FP32, tag="lrr")
            nc.scalar.activation(rr_[:], var_[:], AF.Exp, bias=zero_row[:],
                                 scale=-0.5)
            bn_ = lns.tile([1, TC], FP32, tag="lbn")
            nc.vector.tensor_tensor(bn_[:], mu_[:], rr_[:], op=AX.mult)
            pra = ps.tile([P, TC], FP32, tag="ps512")
            nc.tensor.matmul(pra[:], ones_row[:], rr_[:], start=True, stop=True)
            prb = ps.tile([P, TC], FP32, tag="ps512")
            nc.tensor.matmul(prb[:], ones_row[:], bn_[:], start=True, stop=True)
            gg = lns.tile([P, NF], FP32, tag="lgg")
            bb = lns.tile([P, NF], FP32, tag="lbb")
            nc.sync.dma_start(gg[:, :, None],
                              io[g_name].rearrange("(f p) o -> p f o", p=P))
            nc.sync.dma_start(bb[:, :, None],
                              io[be_name].rearrange("(f p) o -> p f o", p=P))
            for f in range(NF):
                sl = slice(f * TC, (f + 1) * TC)
                xt = get_x(f)
                tt = scr.tile([P, TC], FP32, tag="scr")
                nc.vector.tensor_tensor(tt[:], xt[:], pra[:], op=AX.mult)
                nc.vector.tensor_tensor(tt[:], tt[:], prb[:], op=AX.subtract)
                nc.scalar.activation(out_tile[:, sl], tt[:], AF.Identity,
                                     bias=bb[:, f:f + 1], scale=gg[:, f:f + 1])

        def get_x2(f):
            tl = scr.tile([P, TC], FP32, tag="scr")
            nc.sync.dma_start(tl[:], a2a_out[f * 128:(f + 1) * 128, :])
            return tl
        c_sb = big.tile([P, NF * TC], FP32, tag="big")
        ln_local(get_x2, "g2", "be2", c_sb)

        h_sb = big.tile([P, NF * TC], FP32, tag="big")
        bf1 = lns.tile([P, NF], FP32, tag="bf1")
        bf2 = lns.tile([P, NF], FP32, tag="bf2")
        nc.sync.dma_start(bf1[:, :, None],
                          io["bf1"].rearrange("(f p) o -> p f o", p=P))
        nc.sync.dma_start(bf2[:, :, None],
                          io["bf2"].rearrange("(f p) o -> p f o", p=P))
        for hq in range(NF):
            w1t = wff.tile([P, NF * 128], FP32, tag="wt")
            nc.sync.dma_start(
                w1t[:, :].rearrange("p (f m) -> p f m", f=NF),
                io["w1"][:, hq * 128:(hq + 1) * 128]
                .rearrange("(f p) m -> p f m", p=P))
            pt = ps.tile([P, TC], FP32, tag="ps512")
            for f in range(NF):
                nc.tensor.matmul(pt[:], w1t[:, f * 128:(f + 1) * 128],
                                 c_sb[:, f * TC:(f + 1) * TC],
                                 start=(f == 0), stop=(f == NF - 1))
            nc.scalar.activation(h_sb[:, hq * TC:(hq + 1) * TC], pt[:],
                                 AF.Identity, bias=bf1[:, hq:hq + 1], scale=1.0)
        x3f = []
        for oq in range(NF):
            w2t = wff.tile([P, NF * 128], FP32, tag="wt")
            nc.sync.dma_start(
                w2t[:, :].rearrange("p (f m) -> p f m", f=NF),
                io["w2"][:, oq * 128:(oq + 1) * 128]
                .rearrange("(f p) m -> p f m", p=P))
            pt = ps.tile([P, TC], FP32, tag="ps512")
            for f in range(NF):
                nc.tensor.matmul(pt[:], w2t[:, f * 128:(f + 1) * 128],
                                 h_sb[:, f * TC:(f + 1) * TC],
                                 start=(f == 0), stop=(f == NF - 1))
            x3 = x3fp.tile([P, TC], FP32, tag="x3f")
            nc.vector.scalar_tensor_tensor(
                x3[:], pt[:], 1.0, c_sb[:, oq * TC:(oq + 1) * TC],
                op0=AX.mult, op1=AX.add)
            nc.scalar.activation(x3[:], x3[:], AF.Identity,
                                 bias=bf2[:, oq:oq + 1], scale=1.0)
            x3f.append(x3)

        y_sb = big.tile([P, NF * TC], FP32, tag="big")
        ln_local(lambda f: x3f[f], "g3", "be3", y_sb)
        for f in range(NF):
            nc.sync.dma_start(io["out"][f * 128:(f + 1) * 128, :],
                              y_sb[:, f * TC:(f + 1) * TC])


def _build(cls1, bidx):
    nc = bacc.Bacc("TRN2", target_bir_lowering=False, debug=False,
                   num_devices=NC)
    n_bnd = max(bidx.values()) + 1 if bidx else 0
    io = {}

    def inp(name, shape, dt=FP32):
        io[name] = nc.dram_tensor(name, shape, dt, kind="ExternalInput").ap()

    inp("xdT", [D, T]); inp("xeT", [D, T]); inp("xd_res", [F, T])
    inp("wq1s", [D, F]); inp("wk1s", [D, F]); inp("wv1s", [D, F])
    inp("wq2s", [D, F], BF16); inp("wk2s", [D, F]); inp("wv2s", [D, F])
    inp("w1", [D, D]); inp("w2", [D, D])
    inp("bf1", [D, 1]); inp("bf2", [D, 1])
    inp("g1s", [F, 1]); inp("be1s", [F, 1])
    inp("g2", [D, 1]); inp("be2", [D, 1]); inp("g3", [D, 1]); inp("be3", [D, 1])
    if n_bnd:
        inp("mbnd", [128, n_bnd * 512], BF16)
        inp("mbndbar", [128, n_bnd * 512], BF16)
    io["out"] = nc.dram_tensor("out", [D, TC], FP32, kind="ExternalOutput").ap()

    with tile.TileContext(nc) as tc:
        _emit(nc, tc, io, cls1, bidx)
    nc.compile()
    return nc


def _classify(mT):
    cls = [[CLEAN] * 2 for _ in range(8)]
    bidx = {}
    for t in range(8):
        for j in range(2):
            sub = mT[128 * t:128 * (t + 1), 512 * j:512 * (j + 1)]
            if sub.max() == 0:
                cls[t][j] = CLEAN
            elif sub.min() == 1:
                cls[t][j] = MASKED
            else:
                cls[t][j] = BOUNDARY
                bidx[(t, j)] = len(bidx)
    return cls, bidx


def kernel(**inputs):
    f32 = np.float32
    bf16 = ml_dtypes.bfloat16
    dec = np.asarray(inputs["dec_input"], f32)
    en = np.asarray(inputs["en_input"], f32)
    lam = np.asarray(inputs["look_ahead_mask"], f32)
    msk2 = np.asarray(inputs["mask"], f32)

    assert np.all(msk2 == 0.0), "cross-attention mask expected to be zero"
    assert np.all((lam == 0.0) | (lam == 1.0)), "mask must be binary"
    assert np.all(lam == lam[0:1]), "mask must be batch-uniform"
    for nm in ("bq1", "bk1", "bv1", "bq2", "bk2", "bv2"):
        assert np.all(np.asarray(inputs[nm]) == 0.0), f"{nm} expected zero"

    mT = np.ascontiguousarray(lam[0, 0].T).astype(f32)  # [k, q]
    cls1, bidx = _classify(mT)
    n_bnd = len(bidx)

    xdT = np.ascontiguousarray(dec.reshape(T, D).T)
    xeT = np.ascontiguousarray(en.reshape(T, D).T)

    mbnd = np.zeros((128, max(n_bnd, 1) * 512), bf16)
    mbndbar = np.zeros((128, max(n_bnd, 1) * 512), bf16)
    for (t, j), sl in bidx.items():
        sub = mT[128 * t:128 * (t + 1), 512 * j:512 * (j + 1)]
        mbnd[:, sl * 512:(sl + 1) * 512] = sub.astype(bf16)
        mbndbar[:, sl * 512:(sl + 1) * 512] = (1.0 - sub).astype(bf16)

    Wq1 = np.asarray(inputs["Wq1"], f32); Wk1 = np.asarray(inputs["Wk1"], f32)
    Wv1 = np.asarray(inputs["Wv1"], f32)
    Wq2 = np.asarray(inputs["Wq2"], f32); Wk2 = np.asarray(inputs["Wk2"], f32)
    Wv2 = np.asarray(inputs["Wv2"], f32)
    scale = f32(1.0) / np.sqrt(f32(HD))

    in_maps = []
    for c in range(NC):
        sl = slice(F * c, F * (c + 1))
        m = {
            "xdT": xdT, "xeT": xeT,
            "xd_res": np.ascontiguousarray(xdT[sl]),
            "wq1s": np.ascontiguousarray(Wq1[:, sl] * scale),
            "wk1s": np.ascontiguousarray(Wk1[:, sl]),
            "wv1s": np.ascontiguousarray(Wv1[:, sl]),
            "wq2s": np.ascontiguousarray(Wq2[:, sl] * scale).astype(bf16),
            "wk2s": np.ascontiguousarray(Wk2[:, sl]),
            "wv2s": np.ascontiguousarray(Wv2[:, sl]),
            "w1": np.asarray(inputs["W1"], f32),
            "w2": np.asarray(inputs["W2"], f32),
            "bf1": np.asarray(inputs["bf1"], f32).reshape(D, 1),
            "bf2": np.asarray(inputs["bf2"], f32).reshape(D, 1),
            "g1s": np.ascontiguousarray(
                np.asarray(inputs["g1"], f32)[sl].reshape(F, 1)),
            "be1s": np.ascontiguousarray(
                np.asarray(inputs["be1"], f32)[sl].reshape(F, 1)),
            "g2": np.asarray(inputs["g2"], f32).reshape(D, 1),
            "be2": np.asarray(inputs["be2"], f32).reshape(D, 1),
            "g3": np.asarray(inputs["g3"], f32).reshape(D, 1),
            "be3": np.asarray(inputs["be3"], f32).reshape(D, 1),
        }
        if n_bnd:
            m["mbnd"] = mbnd
            m["mbndbar"] = mbndbar
        in_maps.append(m)

    global _LAST_NC, _LAST_IN_MAPS
    nc = _build(cls1, bidx)
    _LAST_NC, _LAST_IN_MAPS = nc, in_maps
    res = bass_utils.run_bass_kernel_spmd(nc, in_maps, core_ids=list(range(NC)))

    outT = np.empty((D, T), f32)
    for c in range(NC):
        outT[:, TC * c:TC * (c + 1)] = res.results[c]["out"]
    return np.ascontiguousarray(outT.T).reshape(B, S, D).astype(np.float32)
